# revision 19
# baseline (speedup 1.0000x reference)
"""Trainium2 Bass kernel for batched multi-head attention.

Problem: B=8, H=8, S=2048, D=64 f32 attention,
  out = softmax(Q K^T / 64**0.25) V  per (b, h).

Sharding: the 64 (b,h) pairs are split 8-per-core across the 8 NeuronCores
(pure data/head parallelism, no collectives).

Per-core algorithm (per head), in the k-partitioned orientation so no large
on-chip transposes are needed:
  - Host pre-transposes Q to [D, S] (d-major, duplicated on-device into
    partitions 64..127) and packs K as [2D, S/2] with even k-chunks in rows
    0..63 and odd chunks in rows 64..127; everything is cast to bf16.
  - scoresT[k, q] = K^T.T @ Q^T in k-chunks of 128 x q-slabs of 512.  Each
    chunk PAIR runs as two K=64 matmuls packed into disjoint 64-row strips
    of the PE array (2x PE throughput at K=64).
  - exp is SPLIT between two engines so the Scalar engine is no longer the
    wall: ACT does half the chunk-pair groups exactly (scale folded into the
    activation; no max subtraction: exp args stay in f32 range); the Vector
    engine does the other half with a Schraudolph-style exp2:
    bits = round(s*A + B) as int16, bit-reinterpreted as bf16 (~1.5% rms on
    those elements, mostly cancelling through the softmax normalization).
  - AV keeps expT as the *moving* operand with V stationary, augmented with
    a ones column so the softmax denominators fall out of the same
    accumulation: PSUM outT[0:64, q] unnormalized, outT[64, q] = sum.
  - Software-pipelined in PROGRAM ORDER (engine queues execute strictly
    in order, so any instruction with unmet deps blocks its whole queue):
    body t = [staged tail work of older heads] + interleaved [QK pair g /
    exp g / AV chunks of slab t-1] + [PSUM->SBUF copy of slab t-2 on ACT].
    exp leads AV by a full slab and every queued instruction's deps are
    satisfied by the time it reaches its engine, keeping the Tensor engine
    gap-free (and therefore in its high DVFS p-state).
  - Softmax tail per head, staged ONE pipeline body per step so nothing
    blocks an engine or DMA queue: sums row -> DRAM, reload
    128-partition-tiled, DVE reciprocal (partition-parallel, free dim kept
    tiny because reciprocal is ~8 cyc/elem), stride-0 DRAM-read broadcast
    to [64, S], normalize multiply on the otherwise-idle GPSIMD engine,
    DMA out in bf16.
  - DMA work is split across both HWDGE rings (SP carries packed-K/Q-lo
    input streams + the output stores; the Activation ring carries the Q-hi
    duplicate, V, and the softmax-tail hops) so neither ring's in-order
    service stalls the input stream at head boundaries.
  - Host transposes the [D, S] bf16 outputs back to [S, D] f32 (free).
"""
import sys

sys.path.insert(0, "/opt/trn_rl_repo")

import math
from collections import defaultdict
from contextlib import ExitStack

import ml_dtypes
import numpy as np

import concourse.bass as bass
import concourse.tile as tile
from concourse import bacc, mybir
from concourse.bass_utils import run_bass_kernel_spmd

B, H, S, D = 8, 8, 2048, 64
N_CORES = 8
HPC = B * H // N_CORES  # heads per core = 8
SCALE = 1.0 / (D**0.5) ** 0.5  # 1 / 64**0.25
PCHUNK = 128  # k rows per chunk
NCHUNK = S // PCHUNK  # 16
SLAB = 512  # q columns per QK matmul / AV moving tile
NSLAB = S // SLAB  # 4
NGROUP = NCHUNK // 2  # chunk pairs per slab = 8
BF16 = mybir.dt.bfloat16
F32 = mybir.dt.float32
I16 = mybir.dt.int16

# Schraudolph fast-exp constants for bf16 output:
#   exp(s*SCALE) = 2^(s*SCALE*log2e) ~= bf16_bits(round(128*(t + 127 - c)))
# with t = s*SCALE*log2e.  c calibrated numerically on the softmax-attention
# output error (flat optimum ~0.055, robust to round-vs-truncate converts).
SCH_C = 0.055
SCH_A = 128.0 * SCALE * math.log2(math.e)
SCH_B = 128.0 * (127.0 - SCH_C)

# chunk-pair groups per slab handled by the Scalar engine (exact exp); the
# rest go to the Vector engine (fast approximate exp).  Interleaved so both
# engines finish each slab's groups around the same time; sized so both
# stay just under the Tensor engine's per-slab time.
ACT_G = (0, 2, 4, 6)

_COMPILED = {}


def build_kernel():
    nc = bacc.Bacc("TRN2", target_bir_lowering=False, debug=False)
    qt = nc.dram_tensor("q_t", [HPC, D, S], BF16, kind="ExternalInput").ap()
    kt = nc.dram_tensor("k_t", [HPC, 2 * D, S // 2], BF16, kind="ExternalInput").ap()
    v = nc.dram_tensor("v", [HPC, S, D], BF16, kind="ExternalInput").ap()
    out = nc.dram_tensor("out_t", [HPC, D, S], BF16, kind="ExternalOutput").ap()
    # DRAM bounce buffers for the cross-partition softmax-denominator move
    s_dram = nc.dram_tensor("s_scratch", [HPC, S], F32).ap()
    r_dram = nc.dram_tensor("r_scratch", [HPC, S], F32).ap()

    with tile.TileContext(nc) as tc, ExitStack() as ctx:
        qk_pool = ctx.enter_context(tc.tile_pool(name="qk", bufs=3))
        v_pool = ctx.enter_context(tc.tile_pool(name="vp", bufs=3))
        exp_pool = ctx.enter_context(tc.tile_pool(name="exp", bufs=2))
        ot_pool = ctx.enter_context(tc.tile_pool(name="ot", bufs=3))
        fin_pool = ctx.enter_context(tc.tile_pool(name="fin", bufs=2))
        rb_pool = ctx.enter_context(tc.tile_pool(name="rb", bufs=2))
        small_pool = ctx.enter_context(tc.tile_pool(name="small", bufs=2))
        const_pool = ctx.enter_context(tc.tile_pool(name="const", bufs=1))
        # PSUM budget: psqk 3 x 2 banks + psav 2 x 1 bank = 8 banks exactly
        psqk_pool = ctx.enter_context(
            tc.tile_pool(name="psqk", bufs=3, space="PSUM")
        )
        psav_pool = ctx.enter_context(
            tc.tile_pool(name="psav", bufs=2, space="PSUM")
        )

        zbias = const_pool.tile([128, 1], F32)
        nc.vector.memset(zbias[:], 0.0)
        # warm the ACT exp table at t=0 so its ~2.7us load overlaps the first
        # input DMAs instead of delaying the first real exp
        warm = const_pool.tile([128, 1], F32)
        nc.scalar.activation(
            warm[:],
            zbias[:],
            mybir.ActivationFunctionType.Exp,
            bias=zbias[:],
            scale=1.0,
        )

        sums_nat: dict[int, object] = {}
        qt_sb: dict[int, object] = {}
        kt_sb: dict[int, object] = {}
        v_aug: dict[int, object] = {}
        ot_sb: dict[int, object] = {}
        r_row: dict[int, object] = {}
        r_bc: dict[int, object] = {}
        o_fin: dict[int, object] = {}

        def load_head(h):
            # duplicate Q^T/K^T into partitions 64..127 so chunk pairs can be
            # row-packed onto the PE; loads split into column-halves so the
            # first QK pairs can start before the whole tile lands
            qt_sb[h] = qk_pool.tile([2 * D, S], BF16, tag="qt", name="qt_sb")
            kt_sb[h] = qk_pool.tile([2 * D, S // 2], BF16, tag="kt", name="kt_sb")
            HS = S // 2
            for piece in range(2):
                kcols = slice(piece * HS // 2, (piece + 1) * HS // 2)
                cols = slice(piece * HS, (piece + 1) * HS)
                nc.sync.dma_start(kt_sb[h][:, kcols], kt[h][:, kcols])
                nc.sync.dma_start(qt_sb[h][0:D, cols], qt[h][:, cols])
                nc.scalar.dma_start(qt_sb[h][D : 2 * D, cols], qt[h][:, cols])
            v_aug[h] = v_pool.tile(
                [PCHUNK, NCHUNK, D + 1], BF16, tag="vaug", name="v_aug"
            )
            nc.sync.dma_start(
                v_aug[h][:, :, 0:D],
                v[h].rearrange("(c p) d -> p c d", p=PCHUNK),
            )
            nc.gpsimd.memset(v_aug[h][:, :, D : D + 1], 1.0)

        # softmax tail, one stage per pipeline body so that by the time each
        # instruction reaches the head of its engine/DMA queue its inputs are
        # already complete (no head-of-line blocking)
        def tail_sums(hh):
            nc.scalar.dma_start(s_dram[hh], ot_sb[hh][D : D + 1, :])

        def tail_reload(hh):
            sums_nat[hh] = small_pool.tile(
                [128, NCHUNK], F32, tag="sums", name="sums_nat"
            )
            nc.scalar.dma_start(
                sums_nat[hh][:], s_dram[hh].rearrange("(c p) -> p c", p=128)
            )

        def tail_recip(hh):
            # partition-parallel reciprocal (recip is ~8 cyc/elem: keep the
            # free dim tiny) followed by the store back to DRAM
            r_nat = small_pool.tile([128, NCHUNK], F32, tag="rnat", name="r_nat")
            nc.vector.reciprocal(r_nat[:], sums_nat[hh][:])
            nc.scalar.dma_start(
                r_dram[hh].rearrange("(c p) -> p c", p=128), r_nat[:]
            )

        def tail_bcast(hh):
            r_bc[hh] = rb_pool.tile([D, S], F32, tag="rb", name="r_bc")
            # partition-broadcast of the reciprocal row via stride-0 DRAM read
            HS = S // 2
            for piece in range(2):
                nc.scalar.dma_start(
                    r_bc[hh][:, piece * HS : (piece + 1) * HS],
                    bass.AP(
                        r_dram.tensor, hh * S + piece * HS, [[0, D], [1, HS]]
                    ),
                )

        def tail_norm(hh):
            o_fin[hh] = fin_pool.tile([D, S], BF16, tag="ofin", name="o_fin")
            nc.gpsimd.tensor_tensor(
                o_fin[hh][:],
                ot_sb[hh][0:D, :],
                r_bc[hh][:],
                op=mybir.AluOpType.mult,
            )

        def tail_out(hh):
            nc.sync.dma_start(out[hh], o_fin[hh][:])

        NT = HPC * NSLAB  # 32 slabs
        post = defaultdict(list)  # body index -> staged tail work
        prev_exp = None  # (expT tile, head) for slab t-1
        pend_copy = None  # (psav tile, head, slab) awaiting PSUM->SBUF copy

        for t in range(NT + 10):
            for fn in post.pop(t, ()):
                fn()
            live = t < NT
            if live:
                h, s = divmod(t, NSLAB)
                if s == 0:
                    if h == 0:
                        load_head(0)
                    if h + 1 < HPC:
                        load_head(h + 1)
                cur_exp = exp_pool.tile(
                    [PCHUNK, NCHUNK, SLAB], BF16, tag="expT", name="cur_exp"
                )
            psav = None
            if prev_exp is not None:
                psav = psav_pool.tile([D + 1, SLAB], F32, tag="psav", name="psav")
            for g in range(NGROUP):
                if live:
                    ps = psqk_pool.tile(
                        [PCHUNK, 2, SLAB], F32, tag="psqk", name="ps"
                    )
                    for half in range(2):
                        base = half * D  # even chunk in rows 0-63, odd in 64-127
                        nc.tensor.matmul(
                            ps[:, half, :],
                            kt_sb[h][
                                base : base + D, g * PCHUNK : (g + 1) * PCHUNK
                            ],
                            qt_sb[h][base : base + D, s * SLAB : (s + 1) * SLAB],
                            start=True,
                            stop=True,
                        )
                    if g in ACT_G:
                        nc.scalar.activation(
                            cur_exp[:, 2 * g : 2 * g + 2, :],
                            ps[:],
                            mybir.ActivationFunctionType.Exp,
                            bias=zbias[:],
                            scale=SCALE,
                        )
                    else:
                        nc.vector.tensor_scalar(
                            cur_exp[:, 2 * g : 2 * g + 2, :].bitcast(I16),
                            ps[:],
                            SCH_A,
                            SCH_B,
                            op0=mybir.AluOpType.mult,
                            op1=mybir.AluOpType.add,
                        )
                if prev_exp is not None:
                    eT, eh = prev_exp
                    for cc in (2 * g, 2 * g + 1):
                        nc.tensor.matmul(
                            psav[:],
                            v_aug[eh][:, cc, :],
                            eT[:, cc, :],
                            start=(cc == 0),
                            stop=(cc == NCHUNK - 1),
                        )
            # PSUM->SBUF copy for slab t-2 on ACT, after this body's exps
            # (its AV finished during body t-1, so it never stalls the queue)
            if pend_copy is not None:
                cp_psav, cp_h, cp_s = pend_copy
                if cp_s == 0:
                    ot_sb[cp_h] = ot_pool.tile(
                        [D + 1, S], F32, tag="ot", name="ot_sb"
                    )
                nc.scalar.activation(
                    ot_sb[cp_h][:, cp_s * SLAB : (cp_s + 1) * SLAB],
                    cp_psav[:],
                    mybir.ActivationFunctionType.Copy,
                )
                if cp_s == NSLAB - 1:
                    post[t + 1].append(lambda hh=cp_h: tail_sums(hh))
                    post[t + 2].append(lambda hh=cp_h: tail_reload(hh))
                    post[t + 3].append(lambda hh=cp_h: tail_recip(hh))
                    post[t + 4].append(lambda hh=cp_h: tail_bcast(hh))
                    post[t + 5].append(lambda hh=cp_h: tail_norm(hh))
                    post[t + 6].append(lambda hh=cp_h: tail_out(hh))
                pend_copy = None
            if psav is not None:
                ph, psl = divmod(t - 1, NSLAB)
                pend_copy = (psav, ph, psl)
            if live:
                prev_exp = (cur_exp, h)
            else:
                prev_exp = None
    nc.compile()
    return nc


def _get_compiled():
    if "nc" not in _COMPILED:
        _COMPILED["nc"] = build_kernel()
    return _COMPILED["nc"]


def _pack_kt(k_heads):
    # [h, S, D] -> d-major [h, D, chunk, 128] -> even chunks in rows 0-63,
    # odd chunks in rows 64-127 of a [h, 2D, S/2] packed layout
    kt_h = k_heads.transpose(0, 2, 1).reshape(HPC, D, NCHUNK, PCHUNK)
    kp = np.concatenate(
        [
            kt_h[:, :, 0::2, :].reshape(HPC, D, S // 2),
            kt_h[:, :, 1::2, :].reshape(HPC, D, S // 2),
        ],
        axis=1,
    )
    return np.ascontiguousarray(kp).astype(ml_dtypes.bfloat16)


def kernel(query, key, value, _want_results=False):
    nc = _get_compiled()
    q = np.asarray(query).reshape(B * H, S, D)
    k = np.asarray(key).reshape(B * H, S, D)
    v = np.asarray(value).reshape(B * H, S, D)
    in_maps = []
    for c in range(N_CORES):
        sl = slice(c * HPC, (c + 1) * HPC)
        in_maps.append(
            {
                "q_t": np.ascontiguousarray(q[sl].transpose(0, 2, 1)).astype(
                    ml_dtypes.bfloat16
                ),
                "k_t": _pack_kt(k[sl]),
                "v": np.ascontiguousarray(v[sl]).astype(ml_dtypes.bfloat16),
            }
        )
    res = run_bass_kernel_spmd(nc, in_maps, core_ids=list(range(N_CORES)))
    out = np.concatenate(
        [
            res.results[c]["out_t"]
            .astype(np.float32)
            .transpose(0, 2, 1)
            .reshape(1, HPC, S, D)
            for c in range(N_CORES)
        ],
        axis=0,
    ).reshape(B, H, S, D)
    if _want_results:
        return out, res
    return out


if __name__ == "__main__":
    rng = np.random.default_rng(0)
    q = rng.standard_normal((B, H, S, D), dtype=np.float32)
    k = rng.standard_normal((B, H, S, D), dtype=np.float32)
    v = rng.standard_normal((B, H, S, D), dtype=np.float32)
    o = kernel(q, k, v)
    print("kernel output", o.shape, o.dtype)


# revision 20
# speedup vs baseline: 1.3041x; 1.3041x over previous
"""Trainium2 Bass kernel for batched multi-head attention.

Problem: B=8, H=8, S=2048, D=64 f32 attention,
  out = softmax(Q K^T / 64**0.25) V  per (b, h).

Sharding: the 64 (b,h) pairs are split 8-per-core across the 8 NeuronCores
(pure data/head parallelism, no collectives).

Per-core algorithm (per head), in the k-partitioned orientation so no large
on-chip transposes are needed:
  - Host pre-transposes Q to [D, S] (d-major, duplicated on-device into
    partitions 64..127) and packs K as [2D, S/2] with even k-chunks in rows
    0..63 and odd chunks in rows 64..127; everything is cast to bf16.
  - scoresT[k, q] = K^T.T @ Q^T in k-chunks of 128 x q-slabs of 512.  Each
    chunk PAIR runs as two K=64 matmuls packed into disjoint 64-row strips
    of the PE array (2x PE throughput at K=64).
  - exp is SPLIT between two engines so the Scalar engine is no longer the
    wall: ACT does half the chunk-pair groups exactly (scale folded into the
    activation; no max subtraction: exp args stay in f32 range); the Vector
    engine does the other half with a Schraudolph-style exp2:
    bits = round(s*A + B) as int16, bit-reinterpreted as bf16 (~1.5% rms on
    those elements, mostly cancelling through the softmax normalization).
  - AV keeps expT as the *moving* operand with V stationary, augmented with
    a ones column so the softmax denominators fall out of the same
    accumulation: PSUM outT[0:64, q] unnormalized, outT[64, q] = sum.
  - Software-pipelined in PROGRAM ORDER (engine queues execute strictly
    in order, so any instruction with unmet deps blocks its whole queue):
    body t = [staged tail work of older heads] + interleaved [QK pair g /
    exp g / AV chunks of slab t-1] + [PSUM->SBUF copy of slab t-2 on ACT].
    exp leads AV by a full slab and every queued instruction's deps are
    satisfied by the time it reaches its engine, keeping the Tensor engine
    gap-free (and therefore in its high DVFS p-state).
  - Softmax tail per head, staged ONE pipeline body per step so nothing
    blocks an engine or DMA queue: sums row -> DRAM, reload
    128-partition-tiled, DVE reciprocal (partition-parallel, free dim kept
    tiny because reciprocal is ~8 cyc/elem), stride-0 DRAM-read broadcast
    to [64, S], normalize multiply on the otherwise-idle GPSIMD engine,
    DMA out in bf16.
  - DMA work is split across both HWDGE rings (SP carries packed-K/Q-lo
    input streams + the output stores; the Activation ring carries the Q-hi
    duplicate, V, and the softmax-tail hops) so neither ring's in-order
    service stalls the input stream at head boundaries.
  - Host transposes the [D, S] bf16 outputs back to [S, D] f32 (free).
"""
import sys

sys.path.insert(0, "/opt/trn_rl_repo")

import math
from collections import defaultdict
from contextlib import ExitStack

import ml_dtypes
import numpy as np

import concourse.bass as bass
import concourse.tile as tile
from concourse import bacc, mybir
from concourse.bass_utils import run_bass_kernel_spmd

B, H, S, D = 8, 8, 2048, 64
N_CORES = 8
HPC = B * H // N_CORES  # heads per core = 8
SCALE = 1.0 / (D**0.5) ** 0.5  # 1 / 64**0.25
PCHUNK = 128  # k rows per chunk
NCHUNK = S // PCHUNK  # 16
SLAB = 512  # q columns per QK matmul / AV moving tile
NSLAB = S // SLAB  # 4
NGROUP = NCHUNK // 2  # chunk pairs per slab = 8
BF16 = mybir.dt.bfloat16
F32 = mybir.dt.float32
I16 = mybir.dt.int16

# Schraudolph fast-exp constants for bf16 output:
#   exp(s*SCALE) = 2^(s*SCALE*log2e) ~= bf16_bits(round(128*(t + 127 - c)))
# with t = s*SCALE*log2e.  c calibrated numerically on the softmax-attention
# output error (flat optimum ~0.055, robust to round-vs-truncate converts).
SCH_C = 0.055
SCH_A = 128.0 * SCALE * math.log2(math.e)
SCH_B = 128.0 * (127.0 - SCH_C)

# chunk-pair groups per slab handled by the Scalar engine (exact exp); the
# rest go to the Vector engine (fast approximate exp).  Interleaved so both
# engines finish each slab's groups around the same time; sized so both
# stay just under the Tensor engine's per-slab time.
ACT_G = (0, 2, 4, 6)

_COMPILED = {}


def build_kernel():
    nc = bacc.Bacc("TRN2", target_bir_lowering=False, debug=False)
    qt = nc.dram_tensor("q_t", [HPC, D, S], BF16, kind="ExternalInput").ap()
    kt = nc.dram_tensor("k_t", [HPC, 2 * D, S // 2], BF16, kind="ExternalInput").ap()
    v = nc.dram_tensor("v", [HPC, S, D], BF16, kind="ExternalInput").ap()
    out = nc.dram_tensor("out_t", [HPC, D, S], BF16, kind="ExternalOutput").ap()
    # DRAM bounce buffers for the cross-partition softmax-denominator move
    s_dram = nc.dram_tensor("s_scratch", [HPC, S], F32).ap()
    r_dram = nc.dram_tensor("r_scratch", [HPC, S], F32).ap()

    with tile.TileContext(nc) as tc, ExitStack() as ctx:
        qk_pool = ctx.enter_context(tc.tile_pool(name="qk", bufs=3))
        v_pool = ctx.enter_context(tc.tile_pool(name="vp", bufs=3))
        exp_pool = ctx.enter_context(tc.tile_pool(name="exp", bufs=2))
        ot_pool = ctx.enter_context(tc.tile_pool(name="ot", bufs=3))
        fin_pool = ctx.enter_context(tc.tile_pool(name="fin", bufs=2))
        rb_pool = ctx.enter_context(tc.tile_pool(name="rb", bufs=2))
        small_pool = ctx.enter_context(tc.tile_pool(name="small", bufs=2))
        const_pool = ctx.enter_context(tc.tile_pool(name="const", bufs=1))
        # PSUM budget: psqk 3 x 2 banks + psav 2 x 1 bank = 8 banks exactly
        psqk_pool = ctx.enter_context(
            tc.tile_pool(name="psqk", bufs=3, space="PSUM")
        )
        psav_pool = ctx.enter_context(
            tc.tile_pool(name="psav", bufs=2, space="PSUM")
        )

        zbias = const_pool.tile([128, 1], F32)
        nc.vector.memset(zbias[:], 0.0)
        # warm the ACT exp table at t=0 so its ~2.7us load overlaps the first
        # input DMAs instead of delaying the first real exp
        warm = const_pool.tile([128, 1], F32)
        nc.scalar.activation(
            warm[:],
            zbias[:],
            mybir.ActivationFunctionType.Exp,
            bias=zbias[:],
            scale=1.0,
        )

        sums_nat: dict[int, object] = {}
        qt_sb: dict[int, object] = {}
        kt_sb: dict[int, object] = {}
        v_aug: dict[int, object] = {}
        ot_sb: dict[int, object] = {}
        r_row: dict[int, object] = {}
        r_bc: dict[int, object] = {}
        o_fin: dict[int, object] = {}

        def load_head(h):
            # duplicate Q^T/K^T into partitions 64..127 so chunk pairs can be
            # row-packed onto the PE; loads split into column-halves so the
            # first QK pairs can start before the whole tile lands
            qt_sb[h] = qk_pool.tile([2 * D, S], BF16, tag="qt", name="qt_sb")
            kt_sb[h] = qk_pool.tile([2 * D, S // 2], BF16, tag="kt", name="kt_sb")
            HS = S // 2
            for piece in range(2):
                kcols = slice(piece * HS // 2, (piece + 1) * HS // 2)
                cols = slice(piece * HS, (piece + 1) * HS)
                nc.sync.dma_start(kt_sb[h][:, kcols], kt[h][:, kcols])
                nc.sync.dma_start(qt_sb[h][0:D, cols], qt[h][:, cols])
                nc.scalar.dma_start(qt_sb[h][D : 2 * D, cols], qt[h][:, cols])
            v_aug[h] = v_pool.tile(
                [PCHUNK, NCHUNK, D + 1], BF16, tag="vaug", name="v_aug"
            )
            nc.sync.dma_start(
                v_aug[h][:, :, 0:D],
                v[h].rearrange("(c p) d -> p c d", p=PCHUNK),
            )
            nc.gpsimd.memset(v_aug[h][:, :, D : D + 1], 1.0)

        # softmax tail, one stage per pipeline body so that by the time each
        # instruction reaches the head of its engine/DMA queue its inputs are
        # already complete (no head-of-line blocking)
        def tail_sums(hh):
            nc.scalar.dma_start(s_dram[hh], ot_sb[hh][D : D + 1, :])

        def tail_reload(hh):
            sums_nat[hh] = small_pool.tile(
                [128, NCHUNK], F32, tag="sums", name="sums_nat"
            )
            nc.scalar.dma_start(
                sums_nat[hh][:], s_dram[hh].rearrange("(p c) -> p c", p=128)
            )

        def tail_recip(hh):
            # partition-parallel reciprocal (recip is ~8 cyc/elem: keep the
            # free dim tiny) followed by the store back to DRAM
            r_nat = small_pool.tile([128, NCHUNK], F32, tag="rnat", name="r_nat")
            nc.vector.reciprocal(r_nat[:], sums_nat[hh][:])
            nc.scalar.dma_start(
                r_dram[hh].rearrange("(p c) -> p c", p=128), r_nat[:]
            )

        def tail_bcast(hh):
            r_bc[hh] = rb_pool.tile([D, S], F32, tag="rb", name="r_bc")
            # partition-broadcast of the reciprocal row via stride-0 DRAM read
            HS = S // 2
            for piece in range(2):
                nc.scalar.dma_start(
                    r_bc[hh][:, piece * HS : (piece + 1) * HS],
                    bass.AP(
                        r_dram.tensor, hh * S + piece * HS, [[0, D], [1, HS]]
                    ),
                )

        def tail_norm(hh):
            o_fin[hh] = fin_pool.tile([D, S], BF16, tag="ofin", name="o_fin")
            nc.gpsimd.tensor_tensor(
                o_fin[hh][:],
                ot_sb[hh][0:D, :],
                r_bc[hh][:],
                op=mybir.AluOpType.mult,
            )

        def tail_out(hh):
            nc.sync.dma_start(out[hh], o_fin[hh][:])

        NT = HPC * NSLAB  # 32 slabs
        post = defaultdict(list)  # body index -> staged tail work
        prev_exp = None  # (expT tile, head) for slab t-1
        pend_copy = None  # (psav tile, head, slab) awaiting PSUM->SBUF copy

        for t in range(NT + 10):
            for fn in post.pop(t, ()):
                fn()
            live = t < NT
            if live:
                h, s = divmod(t, NSLAB)
                if s == 0:
                    if h == 0:
                        load_head(0)
                    if h + 1 < HPC:
                        load_head(h + 1)
                cur_exp = exp_pool.tile(
                    [PCHUNK, NCHUNK, SLAB], BF16, tag="expT", name="cur_exp"
                )
            psav = None
            if prev_exp is not None:
                psav = psav_pool.tile([D + 1, SLAB], F32, tag="psav", name="psav")
            for g in range(NGROUP):
                if live:
                    ps = psqk_pool.tile(
                        [PCHUNK, 2, SLAB], F32, tag="psqk", name="ps"
                    )
                    for half in range(2):
                        base = half * D  # even chunk in rows 0-63, odd in 64-127
                        nc.tensor.matmul(
                            ps[:, half, :],
                            kt_sb[h][
                                base : base + D, g * PCHUNK : (g + 1) * PCHUNK
                            ],
                            qt_sb[h][base : base + D, s * SLAB : (s + 1) * SLAB],
                            start=True,
                            stop=True,
                        )
                    if g in ACT_G:
                        nc.scalar.activation(
                            cur_exp[:, 2 * g : 2 * g + 2, :],
                            ps[:],
                            mybir.ActivationFunctionType.Exp,
                            bias=zbias[:],
                            scale=SCALE,
                        )
                    else:
                        nc.vector.tensor_scalar(
                            cur_exp[:, 2 * g : 2 * g + 2, :].bitcast(I16),
                            ps[:],
                            SCH_A,
                            SCH_B,
                            op0=mybir.AluOpType.mult,
                            op1=mybir.AluOpType.add,
                        )
                if prev_exp is not None:
                    eT, eh = prev_exp
                    for cc in (2 * g, 2 * g + 1):
                        nc.tensor.matmul(
                            psav[:],
                            v_aug[eh][:, cc, :],
                            eT[:, cc, :],
                            start=(cc == 0),
                            stop=(cc == NCHUNK - 1),
                        )
            # PSUM->SBUF copy for slab t-2 on ACT, after this body's exps
            # (its AV finished during body t-1, so it never stalls the queue)
            if pend_copy is not None:
                cp_psav, cp_h, cp_s = pend_copy
                if cp_s == 0:
                    ot_sb[cp_h] = ot_pool.tile(
                        [D + 1, S], F32, tag="ot", name="ot_sb"
                    )
                nc.scalar.activation(
                    ot_sb[cp_h][:, cp_s * SLAB : (cp_s + 1) * SLAB],
                    cp_psav[:],
                    mybir.ActivationFunctionType.Copy,
                )
                if cp_s == NSLAB - 1:
                    post[t + 1].append(lambda hh=cp_h: tail_sums(hh))
                    post[t + 2].append(lambda hh=cp_h: tail_reload(hh))
                    post[t + 3].append(lambda hh=cp_h: tail_recip(hh))
                    post[t + 4].append(lambda hh=cp_h: tail_bcast(hh))
                    post[t + 5].append(lambda hh=cp_h: tail_norm(hh))
                    post[t + 6].append(lambda hh=cp_h: tail_out(hh))
                pend_copy = None
            if psav is not None:
                ph, psl = divmod(t - 1, NSLAB)
                pend_copy = (psav, ph, psl)
            if live:
                prev_exp = (cur_exp, h)
            else:
                prev_exp = None
    nc.compile()
    return nc


def _get_compiled():
    if "nc" not in _COMPILED:
        _COMPILED["nc"] = build_kernel()
    return _COMPILED["nc"]


def _pack_kt(k_heads):
    # [h, S, D] -> d-major [h, D, chunk, 128] -> even chunks in rows 0-63,
    # odd chunks in rows 64-127 of a [h, 2D, S/2] packed layout
    kt_h = k_heads.transpose(0, 2, 1).reshape(HPC, D, NCHUNK, PCHUNK)
    kp = np.concatenate(
        [
            kt_h[:, :, 0::2, :].reshape(HPC, D, S // 2),
            kt_h[:, :, 1::2, :].reshape(HPC, D, S // 2),
        ],
        axis=1,
    )
    return np.ascontiguousarray(kp).astype(ml_dtypes.bfloat16)


def kernel(query, key, value, _want_results=False):
    nc = _get_compiled()
    q = np.asarray(query).reshape(B * H, S, D)
    k = np.asarray(key).reshape(B * H, S, D)
    v = np.asarray(value).reshape(B * H, S, D)
    in_maps = []
    for c in range(N_CORES):
        sl = slice(c * HPC, (c + 1) * HPC)
        in_maps.append(
            {
                "q_t": np.ascontiguousarray(q[sl].transpose(0, 2, 1)).astype(
                    ml_dtypes.bfloat16
                ),
                "k_t": _pack_kt(k[sl]),
                "v": np.ascontiguousarray(v[sl]).astype(ml_dtypes.bfloat16),
            }
        )
    res = run_bass_kernel_spmd(nc, in_maps, core_ids=list(range(N_CORES)))
    out = np.concatenate(
        [
            res.results[c]["out_t"]
            .astype(np.float32)
            .transpose(0, 2, 1)
            .reshape(1, HPC, S, D)
            for c in range(N_CORES)
        ],
        axis=0,
    ).reshape(B, H, S, D)
    if _want_results:
        return out, res
    return out


if __name__ == "__main__":
    rng = np.random.default_rng(0)
    q = rng.standard_normal((B, H, S, D), dtype=np.float32)
    k = rng.standard_normal((B, H, S, D), dtype=np.float32)
    v = rng.standard_normal((B, H, S, D), dtype=np.float32)
    o = kernel(q, k, v)
    print("kernel output", o.shape, o.dtype)
